# revision 13
# baseline (speedup 1.0000x reference)
"""Trainium2 Bass kernel for nn_BeliefModuleOld (segment_reduce).

Reference semantics per batch element b and treat type tt:
  valid[t] = (vision[b,t] != 0) and (max(visible_treats[b,t,tt,0:5]) > 0.5)
  out[b,tt,:] = visible_treats[b, last valid t, tt, :]  (or [0,0,0,0,0,1] if none)

Strategy: pure data-parallel over batch, 8 cores. Layout A: batch elements
live on SBUF partitions (P=125 used) and along the free dim (F per
partition). Per tile:
  - DMA x [P, F,5,2,6] f32 and v [P, F,5] i32 (contiguous per partition)
  - hm[t,tt] = max over d<5 of x (tensor_max tree on DVE)
  - valid = (hm > 0.5) * vision  (scalar_tensor_tensor)
  - out initialized to the default vector, then for t=0..4 ascending
    copy_predicated(out, valid[t] broadcast over d, x[t]) -- last valid wins
  - DMA out [P, F,2,6]
"""

import numpy as np

import concourse.bass as bass
import concourse.bacc as bacc
import concourse.tile as tile
from concourse import mybir
from concourse.alu_op_type import AluOpType
from concourse.bass_utils import run_bass_kernel_spmd

B, T, NT, D = 1_000_000, 5, 2, 6
NCORES = 8
BC = B // NCORES  # 125,000 per core
P = 125           # SBUF partitions used
F = 125           # batch elements per partition per tile
NTILES = BC // (P * F)  # 8 tiles, exact


def _copy_predicated(eng, out, mask, data):
    # Same as BassVectorEngine.copy_predicated but with opt=False lowering so
    # the three operand APs keep identical [p, f, nt, d] structure (the
    # broadcast mask AP cannot merge dims; unoptimized APs keep the sim's
    # np.where shapes aligned and the HW element streams in lockstep).
    return eng.add_instruction(
        mybir.InstCopyPredicated(
            name=f"I-{eng.bass.next_id()}",
            ins=[eng.lower_ap(mask, opt=False), eng.lower_ap(data, opt=False)],
            outs=[eng.lower_ap(out, opt=False)],
        )
    )


def build_nc(bc=BC, p=P, f=F, reps=1, mode="full"):
    ntiles = bc // (p * f)
    assert p * f * ntiles == bc, (bc, p, f)
    nc = bacc.Bacc("TRN2", target_bir_lowering=False)

    x = nc.dram_tensor("x", [bc, T, NT, D], mybir.dt.float32, kind="ExternalInput")
    v = nc.dram_tensor("v", [bc, T], mybir.dt.int32, kind="ExternalInput")
    o = nc.dram_tensor("o", [bc, NT, D], mybir.dt.float32, kind="ExternalOutput")

    # [ntiles, p, per-partition-contiguous block]
    xr = x[:].rearrange("(n p f) t nt d -> n p (f t nt d)", p=p, f=f)
    vr = v[:].rearrange("(n p f) t -> n p (f t)", p=p, f=f)
    orr = o[:].rearrange("(n p f) nt d -> n p (f nt d)", p=p, f=f)

    fdt = mybir.dt.float32

    with tile.TileContext(nc) as tc:
        with (
            tc.tile_pool(name="xs", bufs=2) as xpool,
            tc.tile_pool(name="vs", bufs=2) as vpool,
            tc.tile_pool(name="os", bufs=2) as opool,
            tc.tile_pool(name="wk", bufs=2) as wpool,
        ):
            if mode == "compute":
                # bench mode: load one tile, run the compute chain reps*ntiles
                # times on resident tiles, store once.
                xt = xpool.tile([p, f, T, NT, D], fdt, tag="x")
                vt = vpool.tile([p, f, T], mybir.dt.int32, tag="v")
                ot = opool.tile([p, f, NT, D], fdt, tag="o")
                nc.sync.dma_start(
                    out=xt[:].rearrange("p f t nt d -> p (f t nt d)"), in_=xr[0]
                )
                nc.sync.dma_start(out=vt[:].rearrange("p f t -> p (f t)"), in_=vr[0])

            for it, i in enumerate(
                [i for _ in range(reps) for i in range(ntiles)]
            ):
                if mode != "compute":
                    xt = xpool.tile([p, f, T, NT, D], fdt, tag="x")
                    vt = vpool.tile([p, f, T], mybir.dt.int32, tag="v")
                    ot = opool.tile([p, f, NT, D], fdt, tag="o")
                    nc.sync.dma_start(
                        out=xt[:].rearrange("p f t nt d -> p (f t nt d)"), in_=xr[i]
                    )
                    nc.sync.dma_start(
                        out=vt[:].rearrange("p f t -> p (f t)"), in_=vr[i]
                    )
                if mode == "dma":
                    # bench mode: DMA traffic only (store a slice of x as o)
                    nc.sync.dma_start(out=orr[i], in_=xt[:, :, 0, :, :])
                    continue

                # out = default = [0,0,0,0,0,1]
                nc.gpsimd.memset(ot[:, :, :, 0:5], 0.0)
                nc.gpsimd.memset(ot[:, :, :, 5:6], 1.0)

                a = wpool.tile([p, f, T, NT], fdt, tag="a")
                bt = wpool.tile([p, f, T, NT], fdt, tag="b")
                c = wpool.tile([p, f, T, NT], fdt, tag="c")
                hm = wpool.tile([p, f, T, NT], fdt, tag="hm")
                # uint8: walrus requires an integer mask dtype for CopyPredicated
                valid = wpool.tile([p, f, T, NT], mybir.dt.uint8, tag="valid")

                nc.vector.tensor_max(a[:], xt[:, :, :, :, 0], xt[:, :, :, :, 1])
                nc.vector.tensor_max(bt[:], xt[:, :, :, :, 2], xt[:, :, :, :, 3])
                nc.vector.tensor_max(c[:], a[:], bt[:])
                nc.vector.tensor_max(hm[:], c[:], xt[:, :, :, :, 4])

                # valid = (hm > 0.5) * vision, vision broadcast over tt
                # (DVE: walrus rejects TensorScalarPtr/TensorTensor on Pool)
                vb = vt[:].unsqueeze(3).broadcast_to((p, f, T, NT))
                nc.vector.scalar_tensor_tensor(
                    out=valid[:],
                    in0=hm[:],
                    scalar=0.5,
                    in1=vb,
                    op0=AluOpType.is_gt,
                    op1=AluOpType.mult,
                )

                # ascending t: last valid timestep wins
                for t in range(T):
                    mask = (
                        valid[:, :, t, :].unsqueeze(3).broadcast_to((p, f, NT, D))
                    )
                    _copy_predicated(nc.vector, ot[:], mask, xt[:, :, t, :, :])

                if mode != "compute":
                    nc.sync.dma_start(
                        out=orr[i], in_=ot[:].rearrange("p f nt d -> p (f nt d)")
                    )

            if mode == "compute":
                nc.sync.dma_start(
                    out=orr[0], in_=ot[:].rearrange("p f nt d -> p (f nt d)")
                )

    nc.compile()
    return nc


_NC = None


def run_spmd(visible_treats: np.ndarray, vision: np.ndarray, **kwargs):
    global _NC
    if _NC is None:
        _NC = build_nc()
    vt = np.ascontiguousarray(visible_treats, dtype=np.float32)
    vi = np.ascontiguousarray(vision, dtype=np.int32)
    in_maps = [
        {
            "x": vt[c * BC : (c + 1) * BC],
            "v": vi[c * BC : (c + 1) * BC],
        }
        for c in range(NCORES)
    ]
    return run_bass_kernel_spmd(_NC, in_maps, core_ids=list(range(NCORES)), **kwargs)


def kernel(visible_treats: np.ndarray, vision: np.ndarray) -> np.ndarray:
    res = run_spmd(visible_treats, vision)
    return np.concatenate([r["o"] for r in res.results], axis=0)


# revision 15
# speedup vs baseline: 7.3911x; 7.3911x over previous
"""Trainium2 Bass kernel for nn_BeliefModuleOld (segment_reduce).

Reference semantics per batch element b and treat type tt:
  valid[t] = (vision[b,t] != 0) and (max(visible_treats[b,t,tt,0:5]) > 0.5)
  out[b,tt,:] = visible_treats[b, last valid t, tt, :]  (or [0,0,0,0,0,1] if none)

Strategy: pure data-parallel over batch, 8 cores. Layout A: batch elements
live on SBUF partitions (P=125 used) and along the free dim (F per
partition). Per tile:
  - DMA x [P, F,5,2,6] f32 and v [P, F,5] i32 (contiguous per partition)
  - hm[t,tt] = max over d<5 of x (tensor_max tree on DVE)
  - valid = (hm > 0.5) * vision  (scalar_tensor_tensor)
  - out initialized to the default vector, then for t=0..4 ascending
    copy_predicated(out, valid[t] broadcast over d, x[t]) -- last valid wins
  - DMA out [P, F,2,6]
"""

import numpy as np

import concourse.bass as bass
import concourse.bacc as bacc
import concourse.tile as tile
from concourse import mybir
from concourse.alu_op_type import AluOpType
from concourse.bass_utils import run_bass_kernel_spmd

B, T, NT, D = 1_000_000, 5, 2, 6
NCORES = 8
BC = B // NCORES  # 125,000 per core
P = 125           # SBUF partitions used
F = 125           # batch elements per partition per tile
NTILES = BC // (P * F)  # 8 tiles, exact


def _copy_predicated(eng, out, mask, data):
    # Same as BassVectorEngine.copy_predicated but with opt=False lowering so
    # the three operand APs keep identical [p, f, nt, d] structure (the
    # broadcast mask AP cannot merge dims; unoptimized APs keep the sim's
    # np.where shapes aligned and the HW element streams in lockstep).
    return eng.add_instruction(
        mybir.InstCopyPredicated(
            name=f"I-{eng.bass.next_id()}",
            ins=[eng.lower_ap(mask, opt=False), eng.lower_ap(data, opt=False)],
            outs=[eng.lower_ap(out, opt=False)],
        )
    )


def build_nc(bc=BC, p=P, f=F, reps=1, mode="full"):
    ntiles = bc // (p * f)
    assert p * f * ntiles == bc, (bc, p, f)
    nc = bacc.Bacc("TRN2", target_bir_lowering=False)

    x = nc.dram_tensor("x", [bc, T, NT, D], mybir.dt.float32, kind="ExternalInput")
    v = nc.dram_tensor("v", [bc, T], mybir.dt.int32, kind="ExternalInput")
    o = nc.dram_tensor("o", [bc, NT, D], mybir.dt.float32, kind="ExternalOutput")

    # [ntiles, p, per-partition-contiguous block]
    xr = x[:].rearrange("(n p f) t nt d -> n p (f t nt d)", p=p, f=f)
    vr = v[:].rearrange("(n p f) t -> n p (f t)", p=p, f=f)
    orr = o[:].rearrange("(n p f) nt d -> n p (f nt d)", p=p, f=f)

    fdt = mybir.dt.float32

    with tile.TileContext(nc) as tc:
        with (
            tc.tile_pool(name="xs", bufs=2) as xpool,
            tc.tile_pool(name="vs", bufs=2) as vpool,
            tc.tile_pool(name="os", bufs=2) as opool,
            tc.tile_pool(name="wk", bufs=2) as wpool,
        ):
            if mode == "compute":
                # bench mode: load one tile, run the compute chain reps*ntiles
                # times on resident tiles, store once.
                xt = xpool.tile([p, f, T, NT, D], fdt, tag="x")
                vt = vpool.tile([p, f, T], mybir.dt.int32, tag="v")
                ot = opool.tile([p, f, NT, D], fdt, tag="o")
                nc.sync.dma_start(
                    out=xt[:].rearrange("p f t nt d -> p (f t nt d)"), in_=xr[0]
                )
                nc.sync.dma_start(out=vt[:].rearrange("p f t -> p (f t)"), in_=vr[0])

            for it, i in enumerate(
                [i for _ in range(reps) for i in range(ntiles)]
            ):
                if mode != "compute":
                    xt = xpool.tile([p, f, T, NT, D], fdt, tag="x")
                    vt = vpool.tile([p, f, T], mybir.dt.int32, tag="v")
                    ot = opool.tile([p, f, NT, D], fdt, tag="o")
                    nc.sync.dma_start(
                        out=xt[:].rearrange("p f t nt d -> p (f t nt d)"), in_=xr[i]
                    )
                    nc.sync.dma_start(
                        out=vt[:].rearrange("p f t -> p (f t)"), in_=vr[i]
                    )
                if mode == "dma":
                    # bench mode: DMA traffic only; store a contiguous chunk
                    # of the x tile with the same shape as the real output
                    xflat = xt[:].rearrange("p f t nt d -> p (f t nt d)")
                    nc.scalar.dma_start(out=orr[i], in_=xflat[:, 0 : f * NT * D])
                    continue

                # out = default = [0,0,0,0,0,1]
                nc.gpsimd.memset(ot[:, :, :, 0:5], 0.0)
                nc.gpsimd.memset(ot[:, :, :, 5:6], 1.0)

                a = wpool.tile([p, f, T, NT], fdt, tag="a")
                bt = wpool.tile([p, f, T, NT], fdt, tag="b")
                c = wpool.tile([p, f, T, NT], fdt, tag="c")
                hm = wpool.tile([p, f, T, NT], fdt, tag="hm")
                # uint8: walrus requires an integer mask dtype for CopyPredicated
                valid = wpool.tile([p, f, T, NT], mybir.dt.uint8, tag="valid")

                nc.vector.tensor_max(a[:], xt[:, :, :, :, 0], xt[:, :, :, :, 1])
                nc.vector.tensor_max(bt[:], xt[:, :, :, :, 2], xt[:, :, :, :, 3])
                nc.vector.tensor_max(c[:], a[:], bt[:])
                nc.vector.tensor_max(hm[:], c[:], xt[:, :, :, :, 4])

                # valid = (hm > 0.5) * vision, vision broadcast over tt
                # (DVE: walrus rejects TensorScalarPtr/TensorTensor on Pool)
                vb = vt[:].unsqueeze(3).broadcast_to((p, f, T, NT))
                nc.vector.scalar_tensor_tensor(
                    out=valid[:],
                    in0=hm[:],
                    scalar=0.5,
                    in1=vb,
                    op0=AluOpType.is_gt,
                    op1=AluOpType.mult,
                )

                # ascending t: last valid timestep wins
                for t in range(T):
                    mask = (
                        valid[:, :, t, :].unsqueeze(3).broadcast_to((p, f, NT, D))
                    )
                    _copy_predicated(nc.vector, ot[:], mask, xt[:, :, t, :, :])

                if mode != "compute":
                    # store on the ACT HWDGE ring so it doesn't queue behind
                    # the next tile's loads on the SP ring
                    nc.scalar.dma_start(
                        out=orr[i], in_=ot[:].rearrange("p f nt d -> p (f nt d)")
                    )

            if mode == "compute":
                nc.sync.dma_start(
                    out=orr[0], in_=ot[:].rearrange("p f nt d -> p (f nt d)")
                )

    nc.compile()
    return nc


_NC = None


def run_spmd(visible_treats: np.ndarray, vision: np.ndarray, **kwargs):
    global _NC
    if _NC is None:
        _NC = build_nc()
    vt = np.ascontiguousarray(visible_treats, dtype=np.float32)
    vi = np.ascontiguousarray(vision, dtype=np.int32)
    in_maps = [
        {
            "x": vt[c * BC : (c + 1) * BC],
            "v": vi[c * BC : (c + 1) * BC],
        }
        for c in range(NCORES)
    ]
    return run_bass_kernel_spmd(_NC, in_maps, core_ids=list(range(NCORES)), **kwargs)


def kernel(visible_treats: np.ndarray, vision: np.ndarray) -> np.ndarray:
    res = run_spmd(visible_treats, vision)
    return np.concatenate([r["o"] for r in res.results], axis=0)


# revision 27
# speedup vs baseline: 7.6025x; 1.0286x over previous
"""Trainium2 Bass kernel for nn_BeliefModuleOld (segment_reduce).

Reference semantics per batch element b and treat type tt:
  valid[t] = (vision[b,t] != 0) and (max(visible_treats[b,t,tt,0:5]) > 0.5)
  out[b,tt,:] = visible_treats[b, last valid t, tt, :]  (or [0,0,0,0,0,1] if none)

Strategy: pure data-parallel over batch, 8 cores. Layout A: batch elements
live on SBUF partitions (P=125 used) and along the free dim (F per
partition). Per tile:
  - DMA x [P, F,5,2,6] f32 and v [P, F,5] i32 (contiguous per partition)
  - hm[t,tt] = max over d<5 of x (tensor_max tree on DVE)
  - valid = (hm > 0.5) * vision  (scalar_tensor_tensor)
  - out initialized to the default vector, then for t=0..4 ascending
    copy_predicated(out, valid[t] broadcast over d, x[t]) -- last valid wins
  - DMA out [P, F,2,6]
"""

import numpy as np

import concourse.bass as bass
import concourse.bacc as bacc
import concourse.tile as tile
from concourse import mybir
from concourse.alu_op_type import AluOpType
from concourse.bass_utils import run_bass_kernel_spmd

B, T, NT, D = 1_000_000, 5, 2, 6
NCORES = 8
BC = B // NCORES  # 125,000 per core
P = 125           # SBUF partitions used
F = 125           # batch elements per partition per tile
NTILES = BC // (P * F)  # 8 tiles, exact


def _copy_predicated(eng, out, mask, data):
    # Same as BassVectorEngine.copy_predicated but with opt=False lowering so
    # the three operand APs keep identical [p, f, nt, d] structure (the
    # broadcast mask AP cannot merge dims; unoptimized APs keep the sim's
    # np.where shapes aligned and the HW element streams in lockstep).
    return eng.add_instruction(
        mybir.InstCopyPredicated(
            name=f"I-{eng.bass.next_id()}",
            ins=[eng.lower_ap(mask, opt=False), eng.lower_ap(data, opt=False)],
            outs=[eng.lower_ap(out, opt=False)],
        )
    )


def build_nc(bc=BC, p=P, f=F, reps=1, mode="full", ring="vact"):
    ntiles = bc // (p * f)
    assert p * f * ntiles == bc, (bc, p, f)
    nc = bacc.Bacc("TRN2", target_bir_lowering=False)

    x = nc.dram_tensor("x", [bc, T, NT, D], mybir.dt.float32, kind="ExternalInput")
    v = nc.dram_tensor("v", [bc, T], mybir.dt.int32, kind="ExternalInput")
    o = nc.dram_tensor("o", [bc, NT, D], mybir.dt.float32, kind="ExternalOutput")

    # [ntiles, p, per-partition-contiguous block]
    xr = x[:].rearrange("(n p f) t nt d -> n p (f t nt d)", p=p, f=f)
    vr = v[:].rearrange("(n p f) t -> n p (f t)", p=p, f=f)
    orr = o[:].rearrange("(n p f) nt d -> n p (f nt d)", p=p, f=f)

    fdt = mybir.dt.float32

    if mode.startswith("load128"):
        # pure-load microbench: x as [128, chunk] tiles from the flat region
        q = 244 if mode == "load128big" else 122
        n128 = (bc // (128 * q))
        xl = x[0 : n128 * 128 * q].rearrange(
            "(n p q) t nt d -> n p (q t nt d)", p=128, q=q
        )
        with tile.TileContext(nc) as tc:
            with tc.tile_pool(name="xs", bufs=2) as xpool:
                for it in range(reps * n128):
                    i = it % n128
                    xt = xpool.tile([128, q * T * NT * D], fdt, tag="x")
                    nc.sync.dma_start(out=xt[:], in_=xl[i])
        nc.compile()
        return nc

    with tile.TileContext(nc) as tc:
        with (
            tc.tile_pool(name="xs", bufs=3) as xpool,
            tc.tile_pool(name="vs", bufs=3) as vpool,
            tc.tile_pool(name="os", bufs=3) as opool,
            tc.tile_pool(name="wk", bufs=2) as wpool,
        ):
            if mode == "compute":
                # bench mode: load one tile, run the compute chain reps*ntiles
                # times on resident tiles, store once.
                xt = xpool.tile([p, f, T, NT, D], fdt, tag="x")
                vt = vpool.tile([p, f, T], mybir.dt.int32, tag="v")
                ot = opool.tile([p, f, NT, D], fdt, tag="o")
                nc.sync.dma_start(
                    out=xt[:].rearrange("p f t nt d -> p (f t nt d)"), in_=xr[0]
                )
                nc.sync.dma_start(out=vt[:].rearrange("p f t -> p (f t)"), in_=vr[0])

            for it, i in enumerate(
                [i for _ in range(reps) for i in range(ntiles)]
            ):
                if mode != "compute":
                    xt = xpool.tile([p, f, T, NT, D], fdt, tag="x")
                    vt = vpool.tile([p, f, T], mybir.dt.int32, tag="v")
                    ot = opool.tile([p, f, NT, D], fdt, tag="o")
                    # ring="fixed": loads on SP ring, stores on ACT ring.
                    # ring="alt": alternate per tile. ring="split": halve the
                    # x-load across both rings. ring="vact": v-load on ACT.
                    xtf = xt[:].rearrange("p f t nt d -> p (f t nt d)")
                    vtf = vt[:].rearrange("p f t -> p (f t)")
                    if ring == "split":
                        h = p // 2
                        nc.sync.dma_start(out=xtf[0:h, :], in_=xr[i][0:h, :])
                        nc.scalar.dma_start(out=xtf[h:p, :], in_=xr[i][h:p, :])
                        nc.sync.dma_start(out=vtf, in_=vr[i])
                    else:
                        ldeng = (
                            nc.sync if (ring != "alt" or it % 2 == 0) else nc.scalar
                        )
                        ldeng.dma_start(out=xtf, in_=xr[i])
                        veng = nc.scalar if ring == "vact" else nc.sync
                        veng.dma_start(out=vtf, in_=vr[i])
                if mode == "dma":
                    # bench mode: DMA traffic only; store a contiguous chunk
                    # of the x tile with the same shape as the real output
                    xflat = xt[:].rearrange("p f t nt d -> p (f t nt d)")
                    nc.scalar.dma_start(out=orr[i], in_=xflat[:, 0 : f * NT * D])
                    continue
                if mode == "load":
                    continue  # bench mode: loads only

                # out = default = [0,0,0,0,0,1]
                nc.gpsimd.memset(ot[:, :, :, 0:5], 0.0)
                nc.gpsimd.memset(ot[:, :, :, 5:6], 1.0)

                a = wpool.tile([p, f, T, NT], fdt, tag="a")
                bt = wpool.tile([p, f, T, NT], fdt, tag="b")
                c = wpool.tile([p, f, T, NT], fdt, tag="c")
                hm = wpool.tile([p, f, T, NT], fdt, tag="hm")
                # uint8: walrus requires an integer mask dtype for CopyPredicated
                valid = wpool.tile([p, f, T, NT], mybir.dt.uint8, tag="valid")

                nc.vector.tensor_max(a[:], xt[:, :, :, :, 0], xt[:, :, :, :, 1])
                nc.vector.tensor_max(bt[:], xt[:, :, :, :, 2], xt[:, :, :, :, 3])
                nc.vector.tensor_max(c[:], a[:], bt[:])
                nc.vector.tensor_max(hm[:], c[:], xt[:, :, :, :, 4])

                # valid = (hm > 0.5) * vision, vision broadcast over tt
                # (DVE: walrus rejects TensorScalarPtr/TensorTensor on Pool)
                vb = vt[:].unsqueeze(3).broadcast_to((p, f, T, NT))
                nc.vector.scalar_tensor_tensor(
                    out=valid[:],
                    in0=hm[:],
                    scalar=0.5,
                    in1=vb,
                    op0=AluOpType.is_gt,
                    op1=AluOpType.mult,
                )

                # ascending t: last valid timestep wins
                for t in range(T):
                    mask = (
                        valid[:, :, t, :].unsqueeze(3).broadcast_to((p, f, NT, D))
                    )
                    _copy_predicated(nc.vector, ot[:], mask, xt[:, :, t, :, :])

                if mode != "compute":
                    # store on the opposite ring from this tile's x-load so it
                    # doesn't queue behind the next tile's big load
                    steng = nc.scalar if (ring != "alt" or it % 2 == 0) else nc.sync
                    steng.dma_start(
                        out=orr[i], in_=ot[:].rearrange("p f nt d -> p (f nt d)")
                    )

            if mode == "compute":
                nc.sync.dma_start(
                    out=orr[0], in_=ot[:].rearrange("p f nt d -> p (f nt d)")
                )

    nc.compile()
    return nc


_NC = None


def run_spmd(visible_treats: np.ndarray, vision: np.ndarray, **kwargs):
    global _NC
    if _NC is None:
        _NC = build_nc()
    if not kwargs.get("trace"):
        # NTFF profiling needs antenv.axon_hooks, absent in this container; a
        # stray BASS_TRACE env var would otherwise crash the run.
        import os

        os.environ.setdefault("BASS_NEVER_TRACE", "1")
    vt = np.ascontiguousarray(visible_treats, dtype=np.float32)
    vi = np.ascontiguousarray(vision, dtype=np.int32)
    in_maps = [
        {
            "x": vt[c * BC : (c + 1) * BC],
            "v": vi[c * BC : (c + 1) * BC],
        }
        for c in range(NCORES)
    ]
    return run_bass_kernel_spmd(_NC, in_maps, core_ids=list(range(NCORES)), **kwargs)


def kernel(visible_treats: np.ndarray, vision: np.ndarray) -> np.ndarray:
    res = run_spmd(visible_treats, vision)
    return np.concatenate([r["o"] for r in res.results], axis=0)
